# revision 3
# baseline (speedup 1.0000x reference)
"""Trainium2 Bass kernel for a 3-layer GCN (Kipf-Welling, symmetric norm,
self-loops) with global add pooling.

Distribution: nodes (graph-aligned contiguous ranges) are sharded across 8
NeuronCores.  Each core owns the aggregation (scatter-add) for its local dst
nodes; the per-layer activations are exchanged with an AllGather so every core
can gather arbitrary source rows with indirect DMA.

Math (matches the jax reference exactly):
    deg  = indeg + 1, dis = deg^-1/2
    Hs   = dis * (H @ W)              (rows scaled by dis)
    agg  = dis_dst * sum_{e:(s->d)} Hs[s]   over edges *including self-loops*
         = sum_e dis_s dis_d (HW)[s] + (HW)[d]/deg_d
    H'   = relu(agg + b)              (no relu on layer 3)
    out  = segment_sum(H3, batch)

Feature-major layout on chip: H^T tiles [128 feats, nodes] so the layer
matmul streams with W as the stationary operand.  The edge scatter-add is a
matmul with an on-the-fly selection matrix S[e, d] = (dst_id[e] == d), built
on the vector engine by comparing per-edge dst ids against an iota row.
"""

import os
import sys
import math

import numpy as np

sys.path.insert(0, "/opt/trn_rl_repo")

import concourse.bass as bass  # noqa: E402
import concourse.bacc as bacc  # noqa: E402
import concourse.tile as tile  # noqa: E402
from concourse import mybir  # noqa: E402
from concourse.bass_utils import run_bass_kernel_spmd  # noqa: E402
from concourse.masks import make_identity  # noqa: E402

P = 128
F32 = mybir.dt.float32
F16 = mybir.dt.float16
I32 = mybir.dt.int32
I16 = mybir.dt.int16
OP = mybir.AluOpType

N_CORES = 8
G_TOTAL = 1000  # graphs in the batch (fixed by the problem)


# ----------------------------------------------------------------------------
# Host-side preprocessing: shard nodes/edges, build gather/selection metadata.
# ----------------------------------------------------------------------------

def _preprocess(x, edge_index, batch, n_cores, G):
    N = x.shape[0]
    src = edge_index[0].astype(np.int64)
    dst = edge_index[1].astype(np.int64)
    batch = batch.astype(np.int64)

    # graph-aligned shard boundaries near equal node counts
    graph_start = np.searchsorted(batch, np.arange(G + 1))  # [G+1], node idx
    bounds = [0]
    for c in range(1, n_cores):
        target = (c * N) // n_cores
        gi = np.searchsorted(graph_start, target)
        lo = graph_start[gi - 1] if gi > 0 else 0
        hi = graph_start[gi] if gi <= G else N
        b = int(hi if (hi - target) <= (target - lo) else lo)
        b = max(b, bounds[-1])  # keep non-decreasing
        bounds.append(b)
    bounds.append(N)
    bounds = np.asarray(bounds, dtype=np.int64)

    shard_sizes = bounds[1:] - bounds[:-1]
    N_loc = int(math.ceil(int(shard_sizes.max()) / P) * P)
    T = N_loc // P

    # normalization (index-derived scalars)
    deg = np.bincount(dst, minlength=N).astype(np.float32) + np.float32(1.0)
    dis = (np.float32(1.0) / np.sqrt(deg)).astype(np.float32)

    # padded-global source row ids (rows of the allgathered Hs table)
    core_of = np.searchsorted(bounds, dst, side="right") - 1
    core_of_src = np.searchsorted(bounds, src, side="right") - 1
    src_pg = core_of_src * N_loc + (src - bounds[core_of_src])

    # src-table quarters: int16 gather indices must stay < 32768
    QC = 2                      # cores per quarter
    NQ = n_cores // QC          # quarters
    QR = QC * N_loc             # rows per quarter
    TB = 2                      # tiles per gather block
    assert QR <= 32767, (QR, N_loc)

    per_core = []
    CPS = 1
    for c in range(n_cores):
        n_real = int(bounds[c + 1] - bounds[c])
        m = core_of == c
        dstl = np.concatenate([dst[m] - bounds[c], np.arange(n_real)])
        srcs = np.concatenate([src_pg[m], c * N_loc + np.arange(n_real)])
        tile_id = dstl // P
        quarter = srcs // QR
        key = tile_id * NQ + quarter
        order = np.argsort(key, kind="stable")
        dstl, srcs, tile_id, quarter, key = (
            dstl[order], srcs[order], tile_id[order], quarter[order],
            key[order])
        counts = np.bincount(key, minlength=T * NQ)
        CPS = max(CPS, int(math.ceil(int(counts.max()) / P)))
        per_core.append((n_real, dstl, srcs, key, counts))

    NCH = T * NQ * CPS          # total chunks per core
    in_maps = []
    g_lo = []
    g_cnt = []
    GW = None
    for c in range(n_cores):
        n_real, dstl, srcs, key, counts = per_core[c]
        # slot grid: edge k of (tile,quarter) group -> chunk k//P, part k%P
        goff = np.concatenate([[0], np.cumsum(counts)])[:-1]
        rank = np.arange(dstl.shape[0]) - goff[key]
        chunk = rank // P
        part = rank % P
        col = key * CPS + chunk            # global chunk column (t, q, c)
        tile_id = key // NQ

        # int16 gather indices in gather-group order:
        # group (tile-block tb, quarter q) -> flat j = (t_loc*CPS+c)*128+p.
        # 16-partition-wrapped within each group, replicated x8 core groups.
        quarter = key % NQ
        tb = tile_id // TB
        t_loc = tile_id % TB
        tbg = np.minimum(TB, T - tb * TB)  # tiles in this block
        block_base = tb * NQ * TB * CPS    # chunk cols before this block
        gcol = block_base + quarter * tbg * CPS + t_loc * CPS + chunk
        flat = gcol * P + part
        idx16 = np.zeros((16, NCH * P // 16), dtype=np.int16)
        idx16[flat % 16, flat // 16] = (srcs % QR).astype(np.int16)
        srcidx = np.tile(idx16, (8, 1))
        dstid = np.full((P, NCH), 1.0e6, dtype=np.float32)
        dstid[part, col] = (dstl - tile_id * P).astype(np.float32)

        dis_loc = np.ones(N_loc, dtype=np.float32)
        dis_loc[:n_real] = dis[bounds[c]:bounds[c + 1]]
        disrep = np.broadcast_to(dis_loc.astype(np.float16), (P, N_loc)).copy()

        xT = np.zeros((P, N_loc), dtype=np.float32)
        xT[:, :n_real] = x[bounds[c]:bounds[c + 1]].T

        bloc = batch[bounds[c]:bounds[c + 1]]
        glo = int(bloc[0]) if n_real > 0 else 0
        gct = int(bloc[-1]) + 1 - glo if n_real > 0 else 0
        g_lo.append(glo)
        g_cnt.append(gct)
        in_maps.append(dict(srcidx=srcidx, dstid=dstid, disrep=disrep, xT=xT,
                            _bloc=bloc - glo, _n_real=n_real))

    GW = max(1, int(math.ceil(max(g_cnt) / P)))
    iota = np.broadcast_to(np.arange(P, dtype=np.float32), (P, P)).copy()
    for c in range(n_cores):
        d = in_maps[c]
        bloc, n_real = d.pop("_bloc"), d.pop("_n_real")
        poolid = np.full((P, T * GW), 1.0e6, dtype=np.float32)
        j = np.arange(n_real)
        for w in range(GW):
            poolid[j % P, (j // P) + w * T] = (bloc - w * P).astype(np.float32)
        d["poolid"] = poolid
        d["iota"] = iota

    cfg = dict(T=T, CPS=CPS, NQ=NQ, QR=QR, TB=TB, GW=GW, N_loc=N_loc,
               n_cores=n_cores)
    return cfg, in_maps, bounds, g_lo, g_cnt


# ----------------------------------------------------------------------------
# Bass program
# ----------------------------------------------------------------------------

def _build_program(cfg):
    T, CPS, GW, N_loc = cfg["T"], cfg["CPS"], cfg["GW"], cfg["N_loc"]
    NQ, QR, TB = cfg["NQ"], cfg["QR"], cfg["TB"]
    n_cores = cfg["n_cores"]
    NCH = T * NQ * CPS
    D, DO = 128, 64
    DOUT = {1: D, 2: D, 3: DO}

    nc = bacc.Bacc(None, num_devices=n_cores)

    xT_d = nc.dram_tensor("xT", [P, N_loc], F32, kind="ExternalInput")
    W_d = {0: nc.dram_tensor("W0", [D, D], F32, kind="ExternalInput"),
           1: nc.dram_tensor("W1", [D, D], F32, kind="ExternalInput"),
           2: nc.dram_tensor("W2", [D, D], F32, kind="ExternalInput"),
           3: nc.dram_tensor("W3", [D, DO], F32, kind="ExternalInput")}
    b_d = {l: nc.dram_tensor(f"b{l}", [P, 1], F32, kind="ExternalInput")
           for l in range(4)}
    srcidx_d = nc.dram_tensor("srcidx", [P, NCH * P // 16], I16,
                              kind="ExternalInput")
    dstid_d = nc.dram_tensor("dstid", [P, NCH], F32, kind="ExternalInput")
    disrep_d = nc.dram_tensor("disrep", [P, N_loc], F16, kind="ExternalInput")
    poolid_d = nc.dram_tensor("poolid", [P, T * GW], F32, kind="ExternalInput")
    iota_d = nc.dram_tensor("iota", [P, P], F32, kind="ExternalInput")
    out_d = nc.dram_tensor("out", [GW * P, DO], F32, kind="ExternalOutput")
    dbg_stage = os.environ.get("GCN_DBG_STAGE", "")
    dbg_d = dbg2_d = None
    if dbg_stage.startswith("h"):
        dbg_d = nc.dram_tensor("dbg", [P, N_loc], F32, kind="ExternalOutput")
    if dbg_stage.startswith("hsf"):
        dbg2_d = nc.dram_tensor("dbg2", [n_cores * N_loc, D], F32,
                                kind="ExternalOutput")

    with tile.TileContext(nc) as tc:
        with tc.tile_pool(name="const", bufs=1) as const, \
             tc.tile_pool(name="hpool", bufs=1) as hpool, \
             tc.tile_pool(name="stage", bufs=3) as stage, \
             tc.tile_pool(name="rpool", bufs=NQ + 1) as rpool, \
             tc.tile_pool(name="spool", bufs=1) as spool, \
             tc.tile_pool(name="dram", bufs=2, space="DRAM") as dram, \
             tc.tile_pool(name="pm", bufs=2, space="PSUM") as pm, \
             tc.tile_pool(name="pt", bufs=2, space="PSUM") as pt, \
             tc.tile_pool(name="pa", bufs=2, space="PSUM") as pa:

            # ---- constants into SBUF
            w_sb = {}
            for l in range(4):
                w = const.tile([D, DOUT.get(l, D) if l else D], F32,
                               name=f"w{l}sb")
                nc.sync.dma_start(out=w[:], in_=W_d[l][:, :])
                w_sb[l] = w
            b_sb = {}
            for l in range(4):
                b = const.tile([P, 1], F32, name=f"b{l}sb")
                nc.sync.dma_start(out=b[:], in_=b_d[l][:, :])
                b_sb[l] = b
            gmin = int(os.environ.get("GCN_MIN", "0"))
            iota_sb = const.tile([P, P], F32, name="iotasb")
            nc.sync.dma_start(out=iota_sb[:], in_=iota_d[:, :])
            iden_sb = const.tile([P, P], F32, name="idensb")
            if not gmin:
                make_identity(nc, iden_sb[:])
            srcidx_sb = const.tile([P, NCH * P // 16], I16, name="srcidxsb")
            nc.sync.dma_start(out=srcidx_sb[:], in_=srcidx_d[:, :])
            dstid_sb = const.tile([P, NCH], F32, name="dstidsb")
            disrep_sb = const.tile([P, N_loc], F16, name="disrepsb")
            poolid_sb = const.tile([P, T * GW], F32, name="poolidsb")
            if not gmin:
                nc.sync.dma_start(out=dstid_sb[:], in_=dstid_d[:, :])
                nc.sync.dma_start(out=disrep_sb[:], in_=disrep_d[:, :])
                nc.sync.dma_start(out=poolid_sb[:], in_=poolid_d[:, :])

            H = hpool.tile([P, N_loc], F32, name="H", tag="ha")

            # Join all const-load DMA sems into the DVE engine clock so later
            # DVE tensor_tensor ops (2 sync-wait slots in the ISA) don't have
            # to carry per-DMA waits themselves.
            if not gmin:
                joiner = const.tile([P, 1], F32, name="joiner")
                for cst in [iota_sb, dstid_sb, disrep_sb, poolid_sb,
                            b_sb[0], b_sb[1], b_sb[2], b_sb[3]]:
                    nc.vector.tensor_copy(out=joiner[:, :1], in_=cst[:, :1])

            # ---- phase 1: M^T = W^T @ Hprev^T, scale by dis, transpose,
            #      write node-major Hs to local DRAM (skipped for l=0/emb).
            def phase1(l, HprevT, HsLocal):
                dout = DOUT.get(l, D)
                nk = (N_loc + 511) // 512
                for k in range(nk):
                    c0 = k * 512
                    cw = min(512, N_loc - c0)
                    if l == 0:
                        # stream x^T chunks from DRAM (saves 6.5MB SBUF)
                        xst = stage.tile([P, 512], F32, name="xst", tag="ms")
                        nc.sync.dma_start(out=xst[:, :cw],
                                          in_=xT_d[:, c0:c0 + cw])
                        rhs_ap = xst[:, :cw]
                    else:
                        rhs_ap = HprevT[:, c0:c0 + cw]
                    mm = pm.tile([P, 512], F32, name="mm", tag="pm")
                    nc.tensor.matmul(mm[:dout, :cw], lhsT=w_sb[l][:, :dout],
                                     rhs=rhs_ap,
                                     start=True, stop=True)
                    if l == 0:
                        nc.vector.tensor_scalar(
                            out=H[:, c0:c0 + cw], in0=mm[:, :cw],
                            scalar1=b_sb[0][:, :], scalar2=None, op0=OP.add)
                        continue
                    ms = stage.tile([P, 512], F32, name="ms", tag="ms")
                    nc.vector.tensor_tensor(
                        out=ms[:dout, :cw], in0=mm[:dout, :cw],
                        in1=disrep_sb[:dout, c0:c0 + cw], op=OP.mult)
                    for tt in range(cw // P):
                        tglob = k * 4 + tt
                        ptt = pt.tile([P, P], F32, name="ptt", tag="pt")
                        nc.tensor.transpose(
                            out=ptt[:, :dout],
                            in_=ms[:dout, tt * P:(tt + 1) * P],
                            identity=iden_sb[:dout, :dout])
                        hs = stage.tile([P, P], F32, name="hs", tag="hs")
                        nc.vector.tensor_copy(out=hs[:, :dout],
                                              in_=ptt[:, :dout])
                        nc.sync.dma_start(
                            out=HsLocal[tglob * P:(tglob + 1) * P, :],
                            in_=hs[:, :dout])

            # ---- phase 2: gather + scatter-add into local dst tiles.
            def phase2(l, HsFull, H3):
                dout = DOUT[l]
                NB = (T + TB - 1) // TB
                for tb in range(NB):
                    tbg = min(TB, T - tb * TB)
                    block_base = tb * NQ * TB * CPS
                    num = tbg * CPS * P
                    Rq = []
                    for q in range(NQ):
                        R = rpool.tile([P, TB * CPS * dout], F32,
                                       name="R", tag="R")
                        c0 = (block_base + q * tbg * CPS) * P // 16
                        gi = phase2.gcount = getattr(phase2, "gcount", 0) + 1
                        real_n = int(os.environ.get("GCN_REAL_GATHERS",
                                                    "99999"))
                        if (int(os.environ.get("GCN_FAKE_GATHER", "0"))
                                or gi > real_n):
                            nc.sync.dma_start(
                                out=R[:, :tbg * CPS * dout],
                                in_=HsFull[q * QR:
                                           q * QR + P * tbg * CPS,
                                           :].rearrange(
                                    "(p c) e -> p (c e)", p=P))
                        else:
                            nc.gpsimd.dma_gather(
                                out_ap=R[:, :tbg * CPS * dout].rearrange(
                                    "p (c e) -> p c e", e=dout),
                                in_ap=HsFull[q * QR:(q + 1) * QR, :],
                                idxs_ap=srcidx_sb[:, c0:c0 + num // 16],
                                num_idxs=num,
                                num_idxs_reg=num,
                                elem_size=dout,
                                single_packet=(num <= 1008))
                        Rq.append(R)
                    if int(os.environ.get("GCN_NO_AGG", "0")):
                        for q in range(NQ):
                            nc.vector.tensor_copy(
                                out=H[:, (tb * NQ + q) % N_loc:
                                      (tb * NQ + q) % N_loc + 1],
                                in_=Rq[q][:, 0:1])
                        continue
                    for tl in range(tbg):
                        phase2_tile(l, tb * TB + tl, tl, Rq, H3)

            def phase2_tile(l, t, tl, Rq, H3):
                dout = DOUT[l]
                nch = NQ * CPS
                S = spool.tile([P, NQ * CPS * P], F32, name="S", tag="S")
                nc.vector.tensor_tensor(
                    out=S[:].rearrange("p (c d) -> p c d", d=P),
                    in0=dstid_sb[:, t * nch:(t + 1) * nch]
                        .unsqueeze(2).broadcast_to([P, nch, P]),
                    in1=iota_sb[:, :].unsqueeze(1).broadcast_to([P, nch, P]),
                    op=OP.is_equal)
                agg = pa.tile([P, P], F32, name="agg", tag="pa")
                for q in range(NQ):
                    for c in range(CPS):
                        k = q * CPS + c
                        nc.tensor.matmul(
                            agg[:dout, :],
                            lhsT=Rq[q][:, (tl * CPS + c) * dout:
                                       (tl * CPS + c + 1) * dout],
                            rhs=S[:, k * P:(k + 1) * P],
                            start=(k == 0), stop=(k == nch - 1))
                tmp = stage.tile([P, P], F32, name="tmp", tag="tmp")
                nc.vector.tensor_tensor(
                    out=tmp[:dout, :], in0=agg[:dout, :],
                    in1=disrep_sb[:dout, t * P:(t + 1) * P], op=OP.mult)
                if l < 3:
                    nc.vector.tensor_scalar(
                        out=H[:, t * P:(t + 1) * P], in0=tmp[:, :],
                        scalar1=b_sb[l][:, :], scalar2=0.0,
                        op0=OP.add, op1=OP.max)
                else:
                    t2 = stage.tile([P, P], F32, name="t2", tag="tmp")
                    nc.vector.tensor_scalar(
                        out=t2[:dout, :], in0=tmp[:dout, :],
                        scalar1=b_sb[3][:dout, :], scalar2=None,
                        op0=OP.add)
                    ptt = pt.tile([P, P], F32, name="ptt2", tag="pt")
                    nc.tensor.transpose(
                        out=ptt[:, :dout], in_=t2[:dout, :],
                        identity=iden_sb[:dout, :dout])
                    nc.vector.tensor_copy(
                        out=H3[:, t * DO:(t + 1) * DO],
                        in_=ptt[:, :dout])

            def dump_dbg(buf, width=None):
                nc.sync.dma_start(out=dbg_d[:, :width] if width else dbg_d[:, :],
                                  in_=buf[:, :width] if width else buf[:, :])

            # ---- the network
            max_layers = int(os.environ.get("GCN_MAX_LAYERS", "3"))
            skip_p1 = int(os.environ.get("GCN_SKIP_P1", "0"))
            if not skip_p1:
                phase1(0, None, None)  # embedding -> H (streams xT from DRAM)
            if dbg_stage == "h0":
                dump_dbg(H)
            H3 = None
            for l in range(1, max_layers + 1):
                dout = DOUT[l]
                HsLocal = dram.tile([N_loc, dout], F32, name=f"hsl{l}",
                                    tag="hsl")
                if skip_p1:
                    for r in range(T):
                        hz = stage.tile([P, P], F32, name="hz", tag="hs")
                        nc.vector.memset(hz[:, :dout], 1.0)
                        nc.sync.dma_start(
                            out=HsLocal[r * P:(r + 1) * P, :],
                            in_=hz[:, :dout])
                else:
                    phase1(l, H, HsLocal)
                HsFull = dram.tile(
                    [n_cores * N_loc, dout], F32, name=f"hsf{l}", tag="hsf",
                    addr_space="Local"
                    if int(os.environ.get("GCN_LOCAL_HSF", "0"))
                    else "Shared")
                if n_cores > 1 and not int(os.environ.get("GCN_NO_CC", "0")):
                    nc.gpsimd.collective_compute(
                        "AllGather", OP.bypass,
                        replica_groups=[list(range(n_cores))],
                        ins=[HsLocal[:, :].opt()],
                        outs=[HsFull[:, :].opt()])
                else:
                    for cc_i in range(n_cores):
                        nc.sync.dma_start(
                            out=HsFull[cc_i * N_loc:(cc_i + 1) * N_loc, :],
                            in_=HsLocal[:, :])
                if dbg_stage == f"hsf{l}":
                    nc.sync.dma_start(out=dbg2_d[:, :], in_=HsFull[:, :])
                if l == 3:
                    H3 = hpool.tile([P, T * DO], F32, name="H3", tag="hx")
                phase2(l, HsFull, H3)
                if dbg_stage == f"h{l}":
                    dump_dbg(H if l < 3 else H3, None if l < 3 else T * DO)

            # ---- global add pool
            if max_layers < 3:
                zz = stage.tile([P, DO], F32, name="zz", tag="ost")
                nc.gpsimd.memset(zz[:], 0.0)
                for w in range(GW):
                    nc.sync.dma_start(out=out_d[w * P:(w + 1) * P, :],
                                      in_=zz[:])
                return nc
            for w in range(GW):
                pp = pt.tile([P, DO], F32, name="pp", tag="pp")
                for t in range(T):
                    sp = spool.tile([P, P], F32, name="sp", tag="sp")
                    nc.vector.tensor_tensor(
                        out=sp[:],
                        in0=poolid_sb[:, w * T + t:w * T + t + 1]
                            .to_broadcast([P, P]),
                        in1=iota_sb[:, :], op=OP.is_equal)
                    nc.tensor.matmul(pp[:], lhsT=sp[:],
                                     rhs=H3[:, t * DO:(t + 1) * DO],
                                     start=(t == 0), stop=(t == T - 1))
                ost = stage.tile([P, DO], F32, name="ost", tag="ost")
                nc.vector.tensor_copy(out=ost[:], in_=pp[:])
                nc.sync.dma_start(out=out_d[w * P:(w + 1) * P, :],
                                  in_=ost[:])

    return nc


# ----------------------------------------------------------------------------
# Driver
# ----------------------------------------------------------------------------

def _run(x, edge_index, batch, W_emb, b_emb, W1, b1, W2, b2, W3, b3,
         G=G_TOTAL, n_cores=N_CORES, trace=False, bench=False):
    x = np.ascontiguousarray(np.asarray(x, dtype=np.float32))
    edge_index = np.ascontiguousarray(np.asarray(edge_index, dtype=np.int64))
    batch_np = np.ascontiguousarray(np.asarray(batch, dtype=np.int64))

    cfg, in_maps, bounds, g_lo, g_cnt = _preprocess(
        x, edge_index, batch_np, n_cores, G)

    def bpad(b):
        v = np.zeros((P, 1), dtype=np.float32)
        b = np.asarray(b, dtype=np.float32).reshape(-1)
        v[:b.shape[0], 0] = b
        return v

    shared = dict(
        W0=np.asarray(W_emb, dtype=np.float32),
        W1=np.asarray(W1, dtype=np.float32),
        W2=np.asarray(W2, dtype=np.float32),
        W3=np.asarray(W3, dtype=np.float32),
        b0=bpad(b_emb), b1=bpad(b1), b2=bpad(b2), b3=bpad(b3))
    for m in in_maps:
        m.update(shared)

    nc = _build_program(cfg)
    nc.finalize()
    if bench:
        import bench as _bench  # test-only sibling module
        br = _bench.bench_program(nc, in_maps, n_cores)
        results = br["results"]
    else:
        res = run_bass_kernel_spmd(nc, in_maps, list(range(n_cores)),
                                   trace=trace)
        results = res.results
        br = res

    out = np.zeros((G, 64), dtype=np.float32)
    for c in range(n_cores):
        oc = np.asarray(results[c]["out"])
        if g_cnt[c] > 0:
            out[g_lo[c]:g_lo[c] + g_cnt[c]] = oc[:g_cnt[c]]
    return out, br


def kernel(**inputs):
    out, _ = _run(G=G_TOTAL, n_cores=N_CORES,
                  trace=bool(int(os.environ.get("GCN_TRACE", "0"))),
                  **inputs)
    return out



# revision 10
# speedup vs baseline: 1.3125x; 1.3125x over previous
"""Trainium2 Bass kernel for a 3-layer GCN (Kipf-Welling, symmetric norm,
self-loops) with global add pooling.

Distribution: nodes (graph-aligned contiguous ranges) are sharded across 8
NeuronCores.  Each core owns the aggregation (scatter-add) for its local dst
nodes; the per-layer dis-scaled activations Hs are exchanged with an
AllGather so every core can gather arbitrary source rows with indirect DMA.

Math (matches the jax reference exactly):
    deg  = indeg + 1, dis = deg^-1/2
    ms   = dis * (H @ W)    (feature-major, node-columns scaled by dis)
    agg  = sum_{e:(s->d)} ms[s]        over real edges (no self-loops)
    H'   = relu((agg + ms[d]) * dis_d + b)     (no relu on layer 3)
    out  = segment_sum(H3, batch)
(The self-loop term (HW)[d]/deg_d == (agg + ms[d])*dis_d 's ms part.)

Feature-major layout on chip: H^T tiles [128 feats, nodes].  The edge
scatter-add is a matmul with an on-the-fly selection matrix
S[e, d] = (dst_id[e] == d), built on the vector engine by comparing
per-edge dst ids against an iota row.  The whole Hs/gather/S path runs in
fp16 (f32 accumulation in PSUM); layer 3 (dout=64) pads W3 to 128 cols so
the gather rows stay 256B-aligned.

Edge slots are ragged: per (dst tile, src quarter) the chunk count is
max over cores of ceil(count/128) (SPMD needs shared loop structure).
"""

import os
import sys
import math

import numpy as np

sys.path.insert(0, "/opt/trn_rl_repo")

import concourse.bass as bass  # noqa: E402
import concourse.bacc as bacc  # noqa: E402
import concourse.tile as tile  # noqa: E402
from concourse import mybir  # noqa: E402
from concourse.bass_utils import run_bass_kernel_spmd  # noqa: E402

P = 128
F32 = mybir.dt.float32
F16 = mybir.dt.float16
I32 = mybir.dt.int32
I16 = mybir.dt.int16
OP = mybir.AluOpType

N_CORES = 8
G_TOTAL = 1000  # graphs in the batch (fixed by the problem)
SENT = 30000.0  # sentinel dst id for padded slots (fp16-exact)


# ----------------------------------------------------------------------------
# Host-side preprocessing: shard nodes/edges, build gather/selection metadata.
# ----------------------------------------------------------------------------

def _preprocess(x, edge_index, batch, n_cores, G):
    N = x.shape[0]
    src = edge_index[0].astype(np.int64)
    dst = edge_index[1].astype(np.int64)
    batch = batch.astype(np.int64)
    TB = int(os.environ.get("GCN_TB", "4"))

    # graph-aligned shard boundaries near equal node counts
    graph_start = np.searchsorted(batch, np.arange(G + 1))  # [G+1], node idx
    bounds = [0]
    for c in range(1, n_cores):
        target = (c * N) // n_cores
        gi = np.searchsorted(graph_start, target)
        lo = graph_start[gi - 1] if gi > 0 else 0
        hi = graph_start[gi] if gi <= G else N
        b = int(hi if (hi - target) <= (target - lo) else lo)
        b = max(b, bounds[-1])  # keep non-decreasing
        bounds.append(b)
    bounds.append(N)
    bounds = np.asarray(bounds, dtype=np.int64)

    shard_sizes = bounds[1:] - bounds[:-1]
    N_loc = int(math.ceil(int(shard_sizes.max()) / P) * P)
    T = N_loc // P

    # normalization (index-derived scalars)
    deg = np.bincount(dst, minlength=N).astype(np.float32) + np.float32(1.0)
    dis = (np.float32(1.0) / np.sqrt(deg)).astype(np.float32)

    # padded-global source row ids (rows of the allgathered Hs table)
    core_of = np.searchsorted(bounds, dst, side="right") - 1
    core_of_src = np.searchsorted(bounds, src, side="right") - 1
    src_pg = core_of_src * N_loc + (src - bounds[core_of_src])

    # src-table quarters: int16 gather indices must stay < 32768
    QC = 2                      # cores per quarter
    NQ = n_cores // QC          # quarters
    QR = QC * N_loc             # rows per quarter
    assert QR <= 32767, (QR, N_loc)

    # per-core (tile, quarter) edge counts -> shared ragged chunk grid
    per_core = []
    cnt_all = np.zeros((n_cores, T, NQ), dtype=np.int64)
    for c in range(n_cores):
        m = core_of == c
        dstl = dst[m] - bounds[c]
        srcs = src_pg[m]
        tile_id = dstl // P
        quarter = srcs // QR
        key = tile_id * NQ + quarter
        order = np.argsort(key, kind="stable")
        dstl, srcs, key = dstl[order], srcs[order], key[order]
        cnt_all[c] = np.bincount(key, minlength=T * NQ).reshape(T, NQ)
        per_core.append((dstl, srcs, key))

    nch = np.ceil(cnt_all.max(axis=0) / P).astype(np.int64)
    nch[:, 0] = np.maximum(nch[:, 0], 1)    # >=1 chunk per dst tile
    ntile = nch.sum(axis=1)                  # [T] chunks per dst tile
    NCH = int(ntile.sum())                   # total chunk columns

    # block (TB dst tiles) x quarter gather-call grid
    NB = (T + TB - 1) // TB
    # chunks per (block, quarter); R-slot offset of tile within its block
    blk_of = np.arange(T) // TB
    bq_chunks = np.zeros((NB, NQ), dtype=np.int64)
    slot_off = np.zeros((T, NQ), dtype=np.int64)   # chunk offset inside Rq
    for t in range(T):
        tb = blk_of[t]
        slot_off[t] = bq_chunks[tb]
        bq_chunks[tb] += nch[t]
    # gather-order column base of (block, quarter) in srcidx
    gq_base = np.zeros((NB, NQ), dtype=np.int64)
    acc = 0
    for tb in range(NB):
        for q in range(NQ):
            gq_base[tb, q] = acc
            acc += bq_chunks[tb, q]
    assert acc == NCH
    # S-build layout: per-tile contiguous dstid columns, (q, c) ordered
    doff = np.zeros(T + 1, dtype=np.int64)
    doff[1:] = np.cumsum(ntile)
    qoff = np.zeros((T, NQ), dtype=np.int64)       # chunk offset inside tile
    qoff[:, 1:] = np.cumsum(nch, axis=1)[:, :-1]

    in_maps = []
    g_lo = []
    g_cnt = []
    for c in range(n_cores):
        dstl, srcs, key = per_core[c]
        tile_id = key // NQ
        quarter = key % NQ
        # rank within (tile, quarter) group
        goff = np.concatenate([[0],
                               np.cumsum(cnt_all[c].reshape(-1))])[:-1]
        rank = np.arange(dstl.shape[0]) - goff[key]
        chunk = rank // P
        part = rank % P
        tl = tile_id % TB
        # gather-order flat slot (for srcidx)
        base_tl = gq_base[blk_of[tile_id], quarter] + slot_off[tile_id,
                                                              quarter]
        gcol = base_tl + chunk
        flat = gcol * P + part
        idx16 = np.zeros((16, NCH * P // 16), dtype=np.int16)
        idx16[flat % 16, flat // 16] = (srcs % QR).astype(np.int16)
        srcidx = np.tile(idx16, (8, 1))
        # S layout (tile-major) dst ids
        dstid = np.full((P, NCH), SENT, dtype=np.float16)
        scol = doff[tile_id] + qoff[tile_id, quarter] + chunk
        dstid[part, scol] = (dstl - tile_id * P).astype(np.float16)

        n_real = int(bounds[c + 1] - bounds[c])
        dis_loc = np.ones(N_loc, dtype=np.float32)
        dis_loc[:n_real] = dis[bounds[c]:bounds[c + 1]]
        disrep = np.broadcast_to(dis_loc.astype(np.float16), (P, N_loc)).copy()

        xT = np.zeros((P, N_loc), dtype=np.float32)
        xT[:, :n_real] = x[bounds[c]:bounds[c + 1]].T

        bloc = batch[bounds[c]:bounds[c + 1]]
        glo = int(bloc[0]) if n_real > 0 else 0
        gct = int(bloc[-1]) + 1 - glo if n_real > 0 else 0
        g_lo.append(glo)
        g_cnt.append(gct)
        in_maps.append(dict(srcidx=srcidx, dstid=dstid, disrep=disrep, xT=xT,
                            _bloc=bloc - glo, _n_real=n_real))

    GW = max(1, int(math.ceil(max(g_cnt) / P)))
    iota = np.broadcast_to(np.arange(P, dtype=np.float16), (P, P)).copy()
    iden = np.eye(P, dtype=np.float16)
    for c in range(n_cores):
        d = in_maps[c]
        bloc, n_real = d.pop("_bloc"), d.pop("_n_real")
        poolid = np.full((P, T * GW), SENT, dtype=np.float16)
        j = np.arange(n_real)
        for w in range(GW):
            poolid[j % P, (j // P) + w * T] = (bloc - w * P).astype(np.float16)
        d["poolid"] = poolid
        d["iota"] = iota
        d["iden"] = iden

    cfg = dict(T=T, NQ=NQ, QR=QR, TB=TB, NB=NB, GW=GW, N_loc=N_loc,
               NCH=NCH, n_cores=n_cores,
               nch=nch, ntile=ntile, bq_chunks=bq_chunks, gq_base=gq_base,
               doff=doff, qoff=qoff, slot_off=slot_off)
    return cfg, in_maps, bounds, g_lo, g_cnt


# ----------------------------------------------------------------------------
# Bass program
# ----------------------------------------------------------------------------

def _build_program(cfg):
    T, GW, N_loc = cfg["T"], cfg["GW"], cfg["N_loc"]
    NQ, QR, TB, NB = cfg["NQ"], cfg["QR"], cfg["TB"], cfg["NB"]
    NCH = cfg["NCH"]
    nch, ntile = cfg["nch"], cfg["ntile"]
    bq_chunks, gq_base = cfg["bq_chunks"], cfg["gq_base"]
    doff, qoff, slot_off = cfg["doff"], cfg["qoff"], cfg["slot_off"]
    n_cores = cfg["n_cores"]
    D, DO = 128, 64
    DPOST = {1: D, 2: D, 3: DO}     # post-aggregation feature width
    NTMAX = int(ntile.max())
    RMAX = int(bq_chunks.max())

    nc = bacc.Bacc(None, num_devices=n_cores)

    xT_d = nc.dram_tensor("xT", [P, N_loc], F32, kind="ExternalInput")
    W_d = {0: nc.dram_tensor("W0", [D, D], F32, kind="ExternalInput"),
           1: nc.dram_tensor("W1", [D, D], F16, kind="ExternalInput"),
           2: nc.dram_tensor("W2", [D, D], F16, kind="ExternalInput"),
           3: nc.dram_tensor("W3", [D, D], F16, kind="ExternalInput")}
    b_d = {l: nc.dram_tensor(f"b{l}", [P, 1], F32, kind="ExternalInput")
           for l in range(4)}
    srcidx_d = nc.dram_tensor("srcidx", [P, NCH * P // 16], I16,
                              kind="ExternalInput")
    dstid_d = nc.dram_tensor("dstid", [P, NCH], F16, kind="ExternalInput")
    disrep_d = nc.dram_tensor("disrep", [P, N_loc], F16, kind="ExternalInput")
    poolid_d = nc.dram_tensor("poolid", [P, T * GW], F16,
                              kind="ExternalInput")
    iota_d = nc.dram_tensor("iota", [P, P], F16, kind="ExternalInput")
    iden_d = nc.dram_tensor("iden", [P, P], F16, kind="ExternalInput")
    out_d = nc.dram_tensor("out", [GW * P, DO], F32, kind="ExternalOutput")

    rbufs = int(os.environ.get("GCN_RBUFS", "9"))
    sbufs = int(os.environ.get("GCN_SBUFS", "2"))

    with tile.TileContext(nc) as tc:
        with tc.tile_pool(name="const", bufs=1) as const, \
             tc.tile_pool(name="hpool", bufs=1) as hpool, \
             tc.tile_pool(name="stage", bufs=3) as stage, \
             tc.tile_pool(name="rpool", bufs=rbufs) as rpool, \
             tc.tile_pool(name="spool", bufs=sbufs) as spool, \
             tc.tile_pool(name="dram", bufs=2, space="DRAM") as dram, \
             tc.tile_pool(name="pm", bufs=2, space="PSUM") as pm, \
             tc.tile_pool(name="pt", bufs=2, space="PSUM") as pt, \
             tc.tile_pool(name="pa", bufs=2, space="PSUM") as pa:

            # ---- constants into SBUF
            w_sb = {}
            for l in range(4):
                w = const.tile([D, D], F32 if l == 0 else F16, name=f"w{l}sb")
                nc.sync.dma_start(out=w[:], in_=W_d[l][:, :])
                w_sb[l] = w
            b_sb = {}
            for l in range(4):
                b = const.tile([P, 1], F32, name=f"b{l}sb")
                nc.sync.dma_start(out=b[:], in_=b_d[l][:, :])
                b_sb[l] = b
            iota_sb = const.tile([P, P], F16, name="iotasb")
            nc.sync.dma_start(out=iota_sb[:], in_=iota_d[:, :])
            iden_sb = const.tile([P, P], F16, name="idensb")
            nc.sync.dma_start(out=iden_sb[:], in_=iden_d[:, :])
            srcidx_sb = const.tile([P, NCH * P // 16], I16, name="srcidxsb")
            nc.sync.dma_start(out=srcidx_sb[:], in_=srcidx_d[:, :])
            dstid_sb = const.tile([P, NCH], F16, name="dstidsb")
            nc.sync.dma_start(out=dstid_sb[:], in_=dstid_d[:, :])
            disrep_sb = const.tile([P, N_loc], F16, name="disrepsb")
            nc.sync.dma_start(out=disrep_sb[:], in_=disrep_d[:, :])
            poolid_sb = const.tile([P, T * GW], F16, name="poolidsb")
            nc.sync.dma_start(out=poolid_sb[:], in_=poolid_d[:, :])

            H = hpool.tile([P, N_loc], F16, name="H", tag="ha")
            MS = hpool.tile([P, N_loc], F16, name="MS", tag="ms")

            # Join const-load DMA sems into the DVE engine clock so later
            # DVE tensor_tensor ops (2 sync-wait slots in the ISA) don't
            # have to carry per-DMA waits themselves.
            joiner = const.tile([P, 1], F32, name="joiner")
            for cst in [iota_sb, dstid_sb, disrep_sb, poolid_sb,
                        b_sb[0], b_sb[1], b_sb[2], b_sb[3]]:
                nc.vector.tensor_copy(out=joiner[:, :1], in_=cst[:, :1])

            # ---- phase 1: M^T = W^T @ Hprev^T, scale by dis into MS,
            #      transpose, write node-major fp16 Hs to local DRAM.
            #      (l=0/emb: H = x @ W_emb + b, no Hs output.)
            def phase1(l, HsLocal):
                nk = (N_loc + 511) // 512
                for k in range(nk):
                    c0 = k * 512
                    cw = min(512, N_loc - c0)
                    if l == 0:
                        # stream x^T chunks from DRAM (saves SBUF)
                        xst = stage.tile([P, 512], F32, name="xst", tag="xs")
                        nc.sync.dma_start(out=xst[:, :cw],
                                          in_=xT_d[:, c0:c0 + cw])
                        rhs_ap = xst[:, :cw]
                    else:
                        rhs_ap = H[:, c0:c0 + cw]
                    mm = pm.tile([P, 512], F32, name="mm", tag="pm")
                    nc.tensor.matmul(mm[:, :cw], lhsT=w_sb[l][:, :],
                                     rhs=rhs_ap, start=True, stop=True)
                    if l == 0:
                        nc.vector.tensor_scalar(
                            out=H[:, c0:c0 + cw], in0=mm[:, :cw],
                            scalar1=b_sb[0][:, :], scalar2=None, op0=OP.add)
                        continue
                    nc.vector.tensor_tensor(
                        out=MS[:, c0:c0 + cw], in0=mm[:, :cw],
                        in1=disrep_sb[:, c0:c0 + cw], op=OP.mult)
                    for tt in range(cw // P):
                        tglob = k * 4 + tt
                        ptt = pt.tile([P, P], F16, name="ptt", tag="pt")
                        nc.tensor.transpose(
                            out=ptt[:, :],
                            in_=MS[:, c0 + tt * P:c0 + (tt + 1) * P],
                            identity=iden_sb[:, :])
                        hs = stage.tile([P, P], F16, name="hs", tag="hs")
                        nc.vector.tensor_copy(out=hs[:, :], in_=ptt[:, :])
                        nc.sync.dma_start(
                            out=HsLocal[tglob * P:(tglob + 1) * P, :],
                            in_=hs[:, :])

            # ---- phase 2: gather + scatter-add into local dst tiles.
            def phase2(l, HsFull, H3):
                fake = int(os.environ.get("GCN_FAKE_GATHER", "0"))
                for tb in range(NB):
                    tbg = min(TB, T - tb * TB)
                    Rq = []
                    for q in range(NQ):
                        nchunk = int(bq_chunks[tb, q])
                        if nchunk == 0:
                            Rq.append(None)
                            continue
                        num = nchunk * P
                        R = rpool.tile([P, RMAX * D], F16, name="R", tag="R")
                        if fake:
                            nc.sync.dma_start(
                                out=R[:, :nchunk * D],
                                in_=HsFull[q * QR:q * QR + num,
                                           :].rearrange(
                                    "(p c) e -> p (c e)", p=P))
                        else:
                            c0 = int(gq_base[tb, q]) * P // 16
                            nc.gpsimd.dma_gather(
                                out_ap=R[:, :nchunk * D].rearrange(
                                    "p (c e) -> p c e", e=D),
                                in_ap=HsFull[q * QR:(q + 1) * QR, :],
                                idxs_ap=srcidx_sb[:, c0:c0 + num // 16],
                                num_idxs=num,
                                num_idxs_reg=num,
                                elem_size=D,
                                single_packet=(num <= 1008))
                        Rq.append(R)
                    for tl in range(tbg):
                        phase2_tile(l, tb * TB + tl, Rq)

            def phase2_tile(l, t, Rq):
                dpost = DPOST[l]
                nt = int(ntile[t])
                S = spool.tile([P, NTMAX * P], F16, name="S", tag="S")
                nc.vector.tensor_tensor(
                    out=S[:, :nt * P].rearrange("p (c d) -> p c d", d=P),
                    in0=dstid_sb[:, doff[t]:doff[t] + nt]
                        .unsqueeze(2).broadcast_to([P, nt, P]),
                    in1=iota_sb[:, :].unsqueeze(1).broadcast_to([P, nt, P]),
                    op=OP.is_equal)
                agg = pa.tile([P, P], F32, name="agg", tag="pa")
                k = 0
                for q in range(NQ):
                    for c in range(int(nch[t, q])):
                        slot = int(slot_off[t, q]) + c
                        scol = int(qoff[t, q]) + c
                        nc.tensor.matmul(
                            agg[:dpost, :],
                            lhsT=Rq[q][:, slot * D:slot * D + dpost],
                            rhs=S[:, scol * P:(scol + 1) * P],
                            start=(k == 0), stop=(k == nt - 1))
                        k += 1
                # (agg + ms) * dis  (+b, relu)
                t1 = stage.tile([P, P], F32, name="t1", tag="t1")
                nc.vector.tensor_tensor(
                    out=t1[:dpost, :], in0=agg[:dpost, :],
                    in1=MS[:dpost, t * P:(t + 1) * P], op=OP.add)
                tmp = stage.tile([P, P], F32, name="tmp", tag="tmp")
                nc.vector.tensor_tensor(
                    out=tmp[:dpost, :], in0=t1[:dpost, :],
                    in1=disrep_sb[:dpost, t * P:(t + 1) * P], op=OP.mult)
                if l < 3:
                    nc.vector.tensor_scalar(
                        out=H[:, t * P:(t + 1) * P], in0=tmp[:, :],
                        scalar1=b_sb[l][:, :], scalar2=0.0,
                        op0=OP.add, op1=OP.max)
                else:
                    t2 = stage.tile([P, P], F16, name="t2", tag="t2")
                    nc.vector.tensor_scalar(
                        out=t2[:dpost, :], in0=tmp[:dpost, :],
                        scalar1=b_sb[3][:dpost, :], scalar2=None,
                        op0=OP.add)
                    ptt = pt.tile([P, P], F16, name="ptt2", tag="pt")
                    nc.tensor.transpose(
                        out=ptt[:, :dpost], in_=t2[:dpost, :],
                        identity=iden_sb[:dpost, :dpost])
                    nc.vector.tensor_copy(
                        out=H3[:, t * DO:(t + 1) * DO],
                        in_=ptt[:, :dpost])

            # ---- the network
            max_layers = int(os.environ.get("GCN_MAX_LAYERS", "3"))
            repeat = int(os.environ.get("GCN_REPEAT", "1"))
            H3 = hpool.tile([P, T * DO], F16, name="H3", tag="hx")
            for _rep in range(repeat):
                phase1(0, None)  # embedding (streams xT from DRAM)
                for l in range(1, max_layers + 1):
                    HsLocal = dram.tile([N_loc, D], F16, name=f"hsl{l}",
                                        tag="hsl")
                    phase1(l, HsLocal)
                    HsFull = dram.tile([n_cores * N_loc, D], F16,
                                       name=f"hsf{l}", tag="hsf",
                                       addr_space="Shared")
                    nc.gpsimd.collective_compute(
                        "AllGather", OP.bypass,
                        replica_groups=[list(range(n_cores))],
                        ins=[HsLocal[:, :].opt()],
                        outs=[HsFull[:, :].opt()])
                    phase2(l, HsFull, H3)

            # ---- global add pool
            if max_layers < 3:
                zz = stage.tile([P, DO], F32, name="zz", tag="ost")
                nc.gpsimd.memset(zz[:], 0.0)
                for w in range(GW):
                    nc.sync.dma_start(out=out_d[w * P:(w + 1) * P, :],
                                      in_=zz[:])
                return nc
            for w in range(GW):
                pp = pt.tile([P, DO], F32, name="pp", tag="pp")
                for t in range(T):
                    sp = spool.tile([P, P], F16, name="sp", tag="sp")
                    nc.vector.tensor_tensor(
                        out=sp[:],
                        in0=poolid_sb[:, w * T + t:w * T + t + 1]
                            .to_broadcast([P, P]),
                        in1=iota_sb[:, :], op=OP.is_equal)
                    nc.tensor.matmul(pp[:], lhsT=sp[:],
                                     rhs=H3[:, t * DO:(t + 1) * DO],
                                     start=(t == 0), stop=(t == T - 1))
                ost = stage.tile([P, DO], F32, name="ost", tag="ost")
                nc.vector.tensor_copy(out=ost[:], in_=pp[:])
                nc.sync.dma_start(out=out_d[w * P:(w + 1) * P, :],
                                  in_=ost[:])

    return nc


# ----------------------------------------------------------------------------
# Driver
# ----------------------------------------------------------------------------

def _run(x, edge_index, batch, W_emb, b_emb, W1, b1, W2, b2, W3, b3,
         G=G_TOTAL, n_cores=N_CORES, trace=False, bench=False):
    x = np.ascontiguousarray(np.asarray(x, dtype=np.float32))
    edge_index = np.ascontiguousarray(np.asarray(edge_index, dtype=np.int64))
    batch_np = np.ascontiguousarray(np.asarray(batch, dtype=np.int64))

    cfg, in_maps, bounds, g_lo, g_cnt = _preprocess(
        x, edge_index, batch_np, n_cores, G)

    def bpad(b):
        v = np.zeros((P, 1), dtype=np.float32)
        b = np.asarray(b, dtype=np.float32).reshape(-1)
        v[:b.shape[0], 0] = b
        return v

    W3p = np.zeros((128, 128), dtype=np.float16)
    W3p[:, :64] = np.asarray(W3, dtype=np.float16)
    shared = dict(
        W0=np.asarray(W_emb, dtype=np.float32),
        W1=np.asarray(W1, dtype=np.float16),
        W2=np.asarray(W2, dtype=np.float16),
        W3=W3p,
        b0=bpad(b_emb), b1=bpad(b1), b2=bpad(b2), b3=bpad(b3))
    for m in in_maps:
        m.update(shared)

    nc = _build_program(cfg)
    nc.finalize()
    if bench:
        import bench as _bench  # test-only sibling module
        br = _bench.bench_program(nc, in_maps, n_cores)
        results = br["results"]
    else:
        res = run_bass_kernel_spmd(nc, in_maps, list(range(n_cores)),
                                   trace=trace)
        results = res.results
        br = res

    out = np.zeros((G, 64), dtype=np.float32)
    for c in range(n_cores):
        oc = np.asarray(results[c]["out"])
        if g_cnt[c] > 0:
            out[g_lo[c]:g_lo[c] + g_cnt[c]] = oc[:g_cnt[c]]
    return out, br


def kernel(**inputs):
    out, _ = _run(G=G_TOTAL, n_cores=N_CORES, **inputs)
    return out
